# revision 5
# baseline (speedup 1.0000x reference)
"""Multi-head self-attention 2d kernel for 8 trn2 NeuronCores.

Sharding: data-parallel over batch B=16 -> 2 batches per core.
Per-core Bass/Tile kernel computes the full attention block for its 2 batches.

Dataflow (per batch, per core):
  xf [C=512 part, N=1024 free]  (C on partitions, 4 tiles of 128)
  q = wq@xf + bq       -> [C, N]   (lhsT = wqT tiles)
  k = wk@xf + bk       -> [C, N]
  vT = xf.T@wvT + bv   -> [N, C]   (lhsT = xf tiles; stored as v_ext [N, 8, 65]
                                    with a ones column at [.., 64])
  per head h:
    eT[j, i] = k_h.T @ q_h        (K=64; two heads auto-packed in PE row groups)
    expT = exp(SCALE * eT)        (ACT, no max subtraction; |SCALE*e| < 8)
    out_u[0:65, i] = v_ext_h.T @ expT   (accumulate over j tiles; row 64 = denom)
    r = 1/denom broadcast over 64 partitions via K=1 matmul with ones
    out_norm[h*64:(h+1)*64, :] = out_u[0:64] * r
  y = gamma*(wo@out_norm + bo) + x
"""

import sys

for _p in ("/opt/trn_rl_repo",):
    if _p not in sys.path:
        sys.path.insert(0, _p)

import numpy as np

import concourse.bass as bass
from concourse import bacc
import concourse.mybir as mybir
import concourse.tile as tile
from concourse.bass_utils import run_bass_kernel_spmd

F32 = mybir.dt.float32
F32R = mybir.dt.float32r
AF = mybir.ActivationFunctionType
ALU = mybir.AluOpType

C = 512
N = 1024
HEADS = 8
HD = C // HEADS  # 64
SCALE = HD ** -0.5
CT = C // 128  # 4 channel tiles
NT = N // 128  # 8 spatial tiles
NCH = N // 512  # 2 free-dim chunks
BPC = 2  # batches per core
NCORES = 8


def _r(ap):
    return ap.bitcast(F32R)


def build_program():
    nc = bacc.Bacc(trn_type="TRN2", target_bir_lowering=False, debug=False,
                  num_devices=NCORES)

    x2 = nc.dram_tensor("x2", [BPC, C, N], F32R, kind="ExternalInput").ap()
    wT = {
        name: nc.dram_tensor(name, [C, C], F32R, kind="ExternalInput").ap()
        for name in ("wqT", "wkT", "wvT", "woT")
    }
    bq_r = nc.dram_tensor("bq_r", [128, CT], F32, kind="ExternalInput").ap()
    bk_r = nc.dram_tensor("bk_r", [128, CT], F32, kind="ExternalInput").ap()
    bo_r = nc.dram_tensor("bo_r", [128, CT], F32, kind="ExternalInput").ap()
    bv = nc.dram_tensor("bv", [C], F32, kind="ExternalInput").ap()
    gamma = nc.dram_tensor("gamma", [1], F32, kind="ExternalInput").ap()
    ones64 = nc.dram_tensor("ones64", [HD], F32R, kind="ExternalInput").ap()
    y2 = nc.dram_tensor("y2", [BPC, C, N], F32, kind="ExternalOutput").ap()

    with tile.TileContext(nc) as tc:
        with (
            tc.tile_pool(name="sb", bufs=1) as sb,
            tc.tile_pool(name="ps", bufs=1, space="PSUM") as ps,
        ):
            # ---- persistent weights / biases ----
            w_sb = {}
            for name in ("wqT", "wkT", "wvT", "woT"):
                tiles = []
                for kc in range(CT):
                    t = sb.tile([128, C], F32R, tag=f"{name}{kc}")
                    nc.sync.dma_start(out=t, in_=wT[name][kc * 128:(kc + 1) * 128, :])
                    tiles.append(t)
                w_sb[name] = tiles

            bq_sb = sb.tile([128, CT], F32, tag="bq")
            nc.sync.dma_start(out=bq_sb, in_=bq_r)
            bk_sb = sb.tile([128, CT], F32, tag="bk")
            nc.sync.dma_start(out=bk_sb, in_=bk_r)
            bo_sb = sb.tile([128, CT], F32, tag="bo")
            nc.sync.dma_start(out=bo_sb, in_=bo_r)
            bv_bc = sb.tile([128, C], F32, tag="bv")
            nc.sync.dma_start(
                out=bv_bc,
                in_=bass.AP(tensor=bv.tensor, offset=bv.offset,
                            ap=[[0, 128]] + list(bv.ap)),
            )
            gam_sb = sb.tile([128, 1], F32, tag="gam")
            nc.sync.dma_start(
                out=gam_sb,
                in_=bass.AP(tensor=gamma.tensor, offset=gamma.offset,
                            ap=[[0, 128]] + list(gamma.ap)),
            )
            ones1 = sb.tile([1, HD], F32R, tag="ones1")
            nc.sync.dma_start(
                out=ones1,
                in_=bass.AP(tensor=ones64.tensor, offset=ones64.offset,
                            ap=[[0, 1]] + list(ones64.ap)))

            for b in range(BPC):
                # ---- load x ----
                xf = []
                for ct in range(CT):
                    t = sb.tile([128, N], F32R, tag=f"xf{ct}", bufs=2)
                    nc.sync.dma_start(out=t, in_=x2[b, ct * 128:(ct + 1) * 128, :])
                    xf.append(t)

                # ---- Q / K projections ----
                q_sb, k_sb = [], []
                for wname, bias_sb, dst in (("wqT", bq_sb, q_sb),
                                            ("wkT", bk_sb, k_sb)):
                    for ot in range(CT):
                        t = sb.tile([128, N], F32R, tag=f"{wname}o{ot}")
                        for nch in range(NCH):
                            p = ps.tile([128, 512], F32, tag="pq", bufs=2)
                            for kc in range(CT):
                                nc.tensor.matmul(
                                    p,
                                    lhsT=_r(w_sb[wname][kc][:, ot * 128:(ot + 1) * 128]),
                                    rhs=_r(xf[kc][:, nch * 512:(nch + 1) * 512]),
                                    start=(kc == 0), stop=(kc == CT - 1),
                                )
                            nc.vector.tensor_scalar_add(
                                t[:, nch * 512:(nch + 1) * 512], p,
                                bias_sb[:, ot:ot + 1])
                        dst.append(t)

                # ---- V projection, transposed, with ones column ----
                v_ext = []
                for nt in range(NT):
                    t = sb.tile([128, HEADS, HD + 1], F32R, tag=f"v{nt}")
                    nc.sync.dma_start(
                        out=t[:, :, HD:HD + 1],
                        in_=bass.AP(tensor=ones64.tensor, offset=ones64.offset,
                                    ap=[[0, 128], [0, HEADS], [1, 1]]))
                    p = ps.tile([128, 512], F32, tag="pq", bufs=2)
                    for kc in range(CT):
                        nc.tensor.matmul(
                            p,
                            lhsT=_r(xf[kc][:, nt * 128:(nt + 1) * 128]),
                            rhs=_r(w_sb["wvT"][kc]),
                            start=(kc == 0), stop=(kc == CT - 1),
                        )
                    nc.vector.tensor_tensor(
                        t[:, :, 0:HD],
                        p.rearrange("p (h d) -> p h d", h=HEADS),
                        bv_bc.rearrange("p (h d) -> p h d", h=HEADS),
                        ALU.add,
                    )
                    v_ext.append(t)

                # ---- attention ----
                on_sb = [sb.tile([128, N], F32R, tag=f"on{ct}", name=f"on{ct}")
                         for ct in range(CT)]
                for hp in range(HEADS // 2):
                    expT = [[], []]
                    for jt in range(NT):
                        for hh in range(2):
                            pe_ps = ps.tile([128, N], F32, tag="pe", bufs=2)
                            for ic in range(NCH):
                                nc.tensor.matmul(
                                    pe_ps[:, ic * 512:(ic + 1) * 512],
                                    lhsT=_r(k_sb[hp][hh * 64:(hh + 1) * 64,
                                                     jt * 128:(jt + 1) * 128]),
                                    rhs=_r(q_sb[hp][hh * 64:(hh + 1) * 64,
                                                    ic * 512:(ic + 1) * 512]),
                                    start=True, stop=True,
                                )
                            e = sb.tile([128, N], F32R, tag="exp", bufs=10)
                            nc.scalar.activation(e, pe_ps, AF.Exp, scale=SCALE)
                            expT[hh].append(e)
                    for hh in range(2):
                        h = 2 * hp + hh
                        pu = ps.tile([128, N], F32, tag="pu", bufs=1)
                        for jt in range(NT):
                            for ic in range(NCH):
                                nc.tensor.matmul(
                                    pu[0:HD + 1, ic * 512:(ic + 1) * 512],
                                    lhsT=_r(v_ext[jt][:, h, :]),
                                    rhs=_r(expT[hh][jt][:, ic * 512:(ic + 1) * 512]),
                                    start=(jt == 0), stop=(jt == NT - 1),
                                )
                        den = sb.tile([1, N], F32R, tag="den", bufs=2)
                        nc.vector.tensor_copy(den, pu[HD:HD + 1, :])
                        r_sb = sb.tile([HD, N], F32, tag="rsb", bufs=2)
                        for ic in range(NCH):
                            rb = ps.tile([HD, 512], F32, tag="pq", bufs=2)
                            nc.tensor.matmul(
                                rb, lhsT=_r(ones1),
                                rhs=_r(den[:, ic * 512:(ic + 1) * 512]),
                                start=True, stop=True,
                            )
                            nc.vector.reciprocal(
                                r_sb[:, ic * 512:(ic + 1) * 512], rb)
                        ct, half = divmod(h, 2)
                        nc.vector.tensor_tensor(
                            on_sb[ct][half * 64:(half + 1) * 64, :],
                            pu[0:HD, :], r_sb, ALU.mult)

                # ---- out projection + residual + store ----
                for ot in range(CT):
                    for nch in range(NCH):
                        p = ps.tile([128, 512], F32, tag="pq", bufs=2)
                        for ct in range(CT):
                            nc.tensor.matmul(
                                p,
                                lhsT=_r(w_sb["woT"][ct][:, ot * 128:(ot + 1) * 128]),
                                rhs=_r(on_sb[ct][:, nch * 512:(nch + 1) * 512]),
                                start=(ct == 0), stop=(ct == CT - 1),
                            )
                        yt = sb.tile([128, 512], F32, tag="y", bufs=4)
                        nc.vector.tensor_scalar(
                            yt, p, bo_sb[:, ot:ot + 1], gam_sb[:, 0:1],
                            ALU.add, ALU.mult)
                        nc.vector.tensor_tensor(
                            yt, yt, xf[ot][:, nch * 512:(nch + 1) * 512].bitcast(F32), ALU.add)
                        nc.sync.dma_start(
                            out=y2[b, ot * 128:(ot + 1) * 128,
                                   nch * 512:(nch + 1) * 512],
                            in_=yt)
    nc.compile()
    return nc


_PROGRAM = None


def _get_program():
    global _PROGRAM
    if _PROGRAM is None:
        _PROGRAM = build_program()
    return _PROGRAM


def kernel(**inputs):
    x = np.ascontiguousarray(inputs["x"], dtype=np.float32)
    B, c, H, W = x.shape
    assert (c, H * W) == (C, N)
    xr = x.reshape(B, C, N)

    wqT = np.ascontiguousarray(inputs["wq"].T.astype(np.float32))
    wkT = np.ascontiguousarray(inputs["wk"].T.astype(np.float32))
    wvT = np.ascontiguousarray(inputs["wv"].T.astype(np.float32))
    woT = np.ascontiguousarray(inputs["wo"].T.astype(np.float32))
    bq_r = np.ascontiguousarray(inputs["bq"].astype(np.float32).reshape(CT, 128).T)
    bk_r = np.ascontiguousarray(inputs["bk"].astype(np.float32).reshape(CT, 128).T)
    bo_r = np.ascontiguousarray(inputs["bo"].astype(np.float32).reshape(CT, 128).T)
    bv = np.ascontiguousarray(inputs["bv"].astype(np.float32))
    gamma = np.ascontiguousarray(inputs["gamma"].astype(np.float32))

    shared = dict(wqT=wqT, wkT=wkT, wvT=wvT, woT=woT,
                  bq_r=bq_r, bk_r=bk_r, bo_r=bo_r, bv=bv, gamma=gamma,
                  ones64=np.ones(HD, dtype=np.float32))
    in_maps = []
    for core in range(NCORES):
        m = dict(shared)
        m["x2"] = np.ascontiguousarray(xr[core * BPC:(core + 1) * BPC])
        in_maps.append(m)

    nc = _get_program()
    res = run_bass_kernel_spmd(nc, in_maps, list(range(NCORES)))
    y = np.concatenate([res.results[i]["y2"] for i in range(NCORES)], axis=0)
    return y.reshape(B, C, H, W).astype(np.float32)


if __name__ == "__main__":
    rng = np.random.default_rng(0)
    ins = {
        "x": rng.standard_normal((16, C, 32, 32), dtype=np.float32),
        "wq": rng.standard_normal((C, C), dtype=np.float32) / 23,
        "bq": rng.standard_normal((C,), dtype=np.float32) / 23,
        "wk": rng.standard_normal((C, C), dtype=np.float32) / 23,
        "bk": rng.standard_normal((C,), dtype=np.float32) / 23,
        "wv": rng.standard_normal((C, C), dtype=np.float32) / 23,
        "bv": rng.standard_normal((C,), dtype=np.float32) / 23,
        "wo": rng.standard_normal((C, C), dtype=np.float32) / 23,
        "bo": rng.standard_normal((C,), dtype=np.float32) / 23,
        "gamma": np.full((1,), 0.1, dtype=np.float32),
    }
    out = kernel(**ins)
    print("kernel ran, out shape", out.shape)


# revision 7
# speedup vs baseline: 1.5863x; 1.5863x over previous
"""Multi-head self-attention 2d kernel for 8 trn2 NeuronCores.

Sharding: data-parallel over batch B=16 -> 2 batches per core.
Per-core Bass/Tile kernel computes the full attention block for its 2 batches.

Dataflow (per batch, per core):
  xf [C=512 part, N=1024 free]  (C on partitions, 4 tiles of 128)
  q = wq@xf + bq       -> [C, N]   (lhsT = wqT tiles)
  k = wk@xf + bk       -> [C, N]
  vT = xf.T@wvT + bv   -> [N, C]   (lhsT = xf tiles; stored as v_ext [N, 8, 65]
                                    with a ones column at [.., 64])
  per head h:
    eT[j, i] = k_h.T @ q_h        (K=64; two heads auto-packed in PE row groups)
    expT = exp(SCALE * eT)        (ACT, no max subtraction; |SCALE*e| < 8)
    out_u[0:65, i] = v_ext_h.T @ expT   (accumulate over j tiles; row 64 = denom)
    r = 1/denom broadcast over 64 partitions via K=1 matmul with ones
    out_norm[h*64:(h+1)*64, :] = out_u[0:64] * r
  y = gamma*(wo@out_norm + bo) + x
"""

import sys

for _p in ("/opt/trn_rl_repo",):
    if _p not in sys.path:
        sys.path.insert(0, _p)

import numpy as np

import concourse.bass as bass
from concourse import bacc
import concourse.mybir as mybir
import concourse.tile as tile
from concourse.bass_utils import run_bass_kernel_spmd

F32 = mybir.dt.float32
F32R = mybir.dt.float32r
AF = mybir.ActivationFunctionType
ALU = mybir.AluOpType

C = 512
N = 1024
HEADS = 8
HD = C // HEADS  # 64
SCALE = HD ** -0.5
CT = C // 128  # 4 channel tiles
NT = N // 128  # 8 spatial tiles
NCH = N // 512  # 2 free-dim chunks
BPC = 2  # batches per core
NCORES = 8


def _r(ap):
    return ap.bitcast(F32R)


def build_program():
    nc = bacc.Bacc(trn_type="TRN2", target_bir_lowering=False, debug=False,
                  num_devices=NCORES)

    x2 = nc.dram_tensor("x2", [BPC, C, N], F32R, kind="ExternalInput").ap()
    wT = {
        name: nc.dram_tensor(name, [C, C], F32R, kind="ExternalInput").ap()
        for name in ("wqT", "wkT", "wvT", "woT")
    }
    bq_r = nc.dram_tensor("bq_r", [128, CT], F32, kind="ExternalInput").ap()
    bk_r = nc.dram_tensor("bk_r", [128, CT], F32, kind="ExternalInput").ap()
    bo_r = nc.dram_tensor("bo_r", [128, CT], F32, kind="ExternalInput").ap()
    bv = nc.dram_tensor("bv", [C], F32, kind="ExternalInput").ap()
    gamma = nc.dram_tensor("gamma", [1], F32, kind="ExternalInput").ap()
    ones64 = nc.dram_tensor("ones64", [HD], F32R, kind="ExternalInput").ap()
    y2 = nc.dram_tensor("y2", [BPC, C, N], F32, kind="ExternalOutput").ap()

    with tile.TileContext(nc) as tc:
        with (
            tc.tile_pool(name="sb", bufs=1) as sb,
            tc.tile_pool(name="ps", bufs=1, space="PSUM") as ps,
        ):
            # ---- persistent weights / biases ----
            w_sb = {}
            for name in ("wqT", "wkT", "wvT", "woT"):
                tiles = []
                for kc in range(CT):
                    t = sb.tile([128, C], F32R, tag=f"{name}{kc}")
                    nc.sync.dma_start(out=t, in_=wT[name][kc * 128:(kc + 1) * 128, :])
                    tiles.append(t)
                w_sb[name] = tiles

            bq_sb = sb.tile([128, CT], F32, tag="bq")
            nc.sync.dma_start(out=bq_sb, in_=bq_r)
            bk_sb = sb.tile([128, CT], F32, tag="bk")
            nc.sync.dma_start(out=bk_sb, in_=bk_r)
            bo_sb = sb.tile([128, CT], F32, tag="bo")
            nc.sync.dma_start(out=bo_sb, in_=bo_r)
            bv_bc = sb.tile([128, C], F32, tag="bv")
            nc.sync.dma_start(
                out=bv_bc,
                in_=bass.AP(tensor=bv.tensor, offset=bv.offset,
                            ap=[[0, 128]] + list(bv.ap)),
            )
            gam_sb = sb.tile([128, 1], F32, tag="gam")
            nc.sync.dma_start(
                out=gam_sb,
                in_=bass.AP(tensor=gamma.tensor, offset=gamma.offset,
                            ap=[[0, 128]] + list(gamma.ap)),
            )
            ones1 = sb.tile([1, HD], F32R, tag="ones1")
            nc.sync.dma_start(
                out=ones1,
                in_=bass.AP(tensor=ones64.tensor, offset=ones64.offset,
                            ap=[[0, 1]] + list(ones64.ap)))
            # v_ext tiles persist across batches; ones row loaded once.
            # Layout [128 j, 65 (d|one), 8 h]: ones row is contiguous 32B.
            v_ext = []
            for nt in range(NT):
                t = sb.tile([128, HD + 1, HEADS], F32R, tag=f"v{nt}",
                            name=f"vext{nt}")
                nc.gpsimd.dma_start(
                    out=t[:, HD, :],
                    in_=bass.AP(tensor=ones64.tensor, offset=ones64.offset,
                                ap=[[0, 128], [1, HEADS]]))
                v_ext.append(t)

            for b in range(BPC):
                # ---- load x ----
                xf = []
                for ct in range(CT):
                    t = sb.tile([128, N], F32R, tag=f"xf{ct}", bufs=2)
                    nc.sync.dma_start(out=t, in_=x2[b, ct * 128:(ct + 1) * 128, :])
                    xf.append(t)

                # ---- Q / K projections ----
                q_sb, k_sb = [], []
                for wname, bias_sb, dst in (("wqT", bq_sb, q_sb),
                                            ("wkT", bk_sb, k_sb)):
                    for ot in range(CT):
                        t = sb.tile([128, N], F32R, tag=f"{wname}o{ot}")
                        for nch in range(NCH):
                            p = ps.tile([128, 512], F32, tag="pq", bufs=2)
                            for kc in range(CT):
                                nc.tensor.matmul(
                                    p,
                                    lhsT=_r(w_sb[wname][kc][:, ot * 128:(ot + 1) * 128]),
                                    rhs=_r(xf[kc][:, nch * 512:(nch + 1) * 512]),
                                    start=(kc == 0), stop=(kc == CT - 1),
                                )
                            nc.vector.tensor_scalar_add(
                                t[:, nch * 512:(nch + 1) * 512], p,
                                bias_sb[:, ot:ot + 1])
                        dst.append(t)

                # ---- V projection, transposed ----
                for nt in range(NT):
                    p = ps.tile([128, 512], F32, tag="pq", bufs=2)
                    for kc in range(CT):
                        nc.tensor.matmul(
                            p,
                            lhsT=_r(xf[kc][:, nt * 128:(nt + 1) * 128]),
                            rhs=_r(w_sb["wvT"][kc]),
                            start=(kc == 0), stop=(kc == CT - 1),
                        )
                    nc.vector.tensor_tensor(
                        v_ext[nt][:, 0:HD, :],
                        p.rearrange("p (h d) -> p d h", h=HEADS),
                        bv_bc.rearrange("p (h d) -> p d h", h=HEADS),
                        ALU.add,
                    )

                # ---- attention ----
                on_sb = [sb.tile([128, N], F32R, tag=f"on{ct}", name=f"on{ct}")
                         for ct in range(CT)]
                for hp in range(HEADS // 2):
                    expT = [[], []]
                    for jt in range(NT):
                        for hh in range(2):
                            pe_ps = ps.tile([128, N], F32, tag="pe", bufs=2)
                            for ic in range(NCH):
                                nc.tensor.matmul(
                                    pe_ps[:, ic * 512:(ic + 1) * 512],
                                    lhsT=_r(k_sb[hp][hh * 64:(hh + 1) * 64,
                                                     jt * 128:(jt + 1) * 128]),
                                    rhs=_r(q_sb[hp][hh * 64:(hh + 1) * 64,
                                                    ic * 512:(ic + 1) * 512]),
                                    start=True, stop=True,
                                )
                            e = sb.tile([128, N], F32R, tag="exp", bufs=10)
                            nc.scalar.activation(e, pe_ps, AF.Exp, scale=SCALE)
                            expT[hh].append(e)
                    for hh in range(2):
                        h = 2 * hp + hh
                        ct, half = divmod(h, 2)
                        for ic in range(NCH):
                            pu = ps.tile([128, 512], F32, tag="pu", bufs=2)
                            for jt in range(NT):
                                nc.tensor.matmul(
                                    pu[0:HD + 1, :],
                                    lhsT=_r(v_ext[jt][:, :, h]),
                                    rhs=_r(expT[hh][jt][:, ic * 512:(ic + 1) * 512]),
                                    start=(jt == 0), stop=(jt == NT - 1),
                                )
                            den = sb.tile([1, 512], F32R, tag="den", bufs=2)
                            nc.vector.tensor_copy(den, pu[HD:HD + 1, :])
                            rb = ps.tile([HD, 512], F32, tag="pq", bufs=2)
                            nc.tensor.matmul(rb, lhsT=_r(ones1), rhs=_r(den),
                                             start=True, stop=True)
                            r_sb = sb.tile([HD, 512], F32, tag="rsb", bufs=2)
                            nc.vector.reciprocal_approx_fast(out=r_sb, in_=rb)
                            nc.vector.tensor_tensor(
                                on_sb[ct][half * 64:(half + 1) * 64,
                                          ic * 512:(ic + 1) * 512],
                                pu[0:HD, :], r_sb, ALU.mult)

                # ---- out projection + residual + store ----
                for ot in range(CT):
                    for nch in range(NCH):
                        p = ps.tile([128, 512], F32, tag="pq", bufs=2)
                        for ct in range(CT):
                            nc.tensor.matmul(
                                p,
                                lhsT=_r(w_sb["woT"][ct][:, ot * 128:(ot + 1) * 128]),
                                rhs=_r(on_sb[ct][:, nch * 512:(nch + 1) * 512]),
                                start=(ct == 0), stop=(ct == CT - 1),
                            )
                        yt = sb.tile([128, 512], F32, tag="y", bufs=4)
                        nc.vector.tensor_scalar(
                            yt, p, bo_sb[:, ot:ot + 1], gam_sb[:, 0:1],
                            ALU.add, ALU.mult)
                        nc.vector.tensor_tensor(
                            yt, yt, xf[ot][:, nch * 512:(nch + 1) * 512].bitcast(F32), ALU.add)
                        nc.gpsimd.dma_start(
                            out=y2[b, ot * 128:(ot + 1) * 128,
                                   nch * 512:(nch + 1) * 512],
                            in_=yt)
    nc.compile()
    return nc


_PROGRAM = None


def _get_program():
    global _PROGRAM
    if _PROGRAM is None:
        _PROGRAM = build_program()
    return _PROGRAM


def kernel(**inputs):
    x = np.ascontiguousarray(inputs["x"], dtype=np.float32)
    B, c, H, W = x.shape
    assert (c, H * W) == (C, N)
    xr = x.reshape(B, C, N)

    wqT = np.ascontiguousarray(inputs["wq"].T.astype(np.float32))
    wkT = np.ascontiguousarray(inputs["wk"].T.astype(np.float32))
    wvT = np.ascontiguousarray(inputs["wv"].T.astype(np.float32))
    woT = np.ascontiguousarray(inputs["wo"].T.astype(np.float32))
    bq_r = np.ascontiguousarray(inputs["bq"].astype(np.float32).reshape(CT, 128).T)
    bk_r = np.ascontiguousarray(inputs["bk"].astype(np.float32).reshape(CT, 128).T)
    bo_r = np.ascontiguousarray(inputs["bo"].astype(np.float32).reshape(CT, 128).T)
    bv = np.ascontiguousarray(inputs["bv"].astype(np.float32))
    gamma = np.ascontiguousarray(inputs["gamma"].astype(np.float32))

    shared = dict(wqT=wqT, wkT=wkT, wvT=wvT, woT=woT,
                  bq_r=bq_r, bk_r=bk_r, bo_r=bo_r, bv=bv, gamma=gamma,
                  ones64=np.ones(HD, dtype=np.float32))
    in_maps = []
    for core in range(NCORES):
        m = dict(shared)
        m["x2"] = np.ascontiguousarray(xr[core * BPC:(core + 1) * BPC])
        in_maps.append(m)

    nc = _get_program()
    res = run_bass_kernel_spmd(nc, in_maps, list(range(NCORES)))
    y = np.concatenate([res.results[i]["y2"] for i in range(NCORES)], axis=0)
    return y.reshape(B, C, H, W).astype(np.float32)


if __name__ == "__main__":
    rng = np.random.default_rng(0)
    ins = {
        "x": rng.standard_normal((16, C, 32, 32), dtype=np.float32),
        "wq": rng.standard_normal((C, C), dtype=np.float32) / 23,
        "bq": rng.standard_normal((C,), dtype=np.float32) / 23,
        "wk": rng.standard_normal((C, C), dtype=np.float32) / 23,
        "bk": rng.standard_normal((C,), dtype=np.float32) / 23,
        "wv": rng.standard_normal((C, C), dtype=np.float32) / 23,
        "bv": rng.standard_normal((C,), dtype=np.float32) / 23,
        "wo": rng.standard_normal((C, C), dtype=np.float32) / 23,
        "bo": rng.standard_normal((C,), dtype=np.float32) / 23,
        "gamma": np.full((1,), 0.1, dtype=np.float32),
    }
    out = kernel(**ins)
    print("kernel ran, out shape", out.shape)
